# revision 25
# baseline (speedup 1.0000x reference)
"""Trainium2 Bass kernel for nn_CascadedSpatialCrossAttention.

Sharding: data-parallel over batch. B=8 batch elements -> 8 NeuronCores,
one batch element per core. Params are replicated. No collectives.

Per-core layout: an image tensor (64ch, 128, 128) is stored with
partition p = c + 64*parity (parity = h % 2), i.e. even rows of channel c
on partition c, odd rows on partition c+64.  This uses all 128 partitions
for elementwise/reduce work and lets conv3x3 taps be K-stacked in pairs
(even+odd source rows share one free-dim offset).

v2: all hot matmuls run in bf16 (4x PE throughput vs fp32):
  - conv3x3 reads a bf16 shadow copy of the feature tile (t_featb)
  - conv PSUM is evicted by the Scalar engine (Copy + accum) to bf16
  - the spatial-weights matmuls use 64-wide replicated lhsT columns so
    their PSUM output is already broadcast across partitions (the old
    ones-matmul replication step is gone)
  - x12 = x2 + k*Sg is never materialized; its two terms enter the
    weights matmul as separate bf16 rhs operands (x11@x2 + (x11*k)@Sg)
Elementwise work is split between Vector (DVE) and GpSimd (Pool) engines.
"""

import sys

sys.path.insert(0, "/opt/trn_rl_repo")

import numpy as np

import concourse.bass as bass
import concourse.bacc as bacc
import concourse.tile as tile
from concourse import mybir
from concourse.masks import make_identity

F32 = mybir.dt.float32
BF16 = mybir.dt.bfloat16
AF = mybir.ActivationFunctionType
ALU = mybir.AluOpType
AX = mybir.AxisListType

G = 4          # groups
C = 64         # channels per group
H = W = 128
J = 64         # row pairs per parity
NCHUNK = 16    # free-dim chunks of 512 (4 rows) per parity
EPS = 1e-5

# row split for DVE/Pool elementwise sharing (Pool is ~2x slower per elem)
RSPL = 42
# chunk split for the final multiply (DVE chunks < CSPL, Pool the rest)
CSPL = 21


def _sigmoid_softmax(nc, sm, vec, n):
    """softmax over vec [1, n] (SBUF, partition 0), using sigmoid-based exp
    (exp(z) = s/(1-s), s = sigmoid(z), z <= 0 after max subtraction)."""
    mx = sm.tile([1, 1], F32, tag="sm_mx")
    nc.vector.tensor_reduce(mx, vec, axis=AX.X, op=ALU.max)
    z = sm.tile([1, n], F32, tag="sm_z")
    nc.vector.tensor_scalar(z, vec, mx, None, op0=ALU.subtract)
    s = sm.tile([1, n], F32, tag="sm_s")
    nc.scalar.activation(s, z, AF.Sigmoid)
    u = sm.tile([1, n], F32, tag="sm_u")
    nc.vector.tensor_scalar(u, s, -1.0, 1.0, op0=ALU.mult, op1=ALU.add)
    r = sm.tile([1, n], F32, tag="sm_r")
    nc.vector.reciprocal(r, u)
    e = sm.tile([1, n], F32, tag="sm_e")
    nc.vector.tensor_tensor(e, s, r, op=ALU.mult)
    se = sm.tile([1, 1], F32, tag="sm_se")
    nc.vector.tensor_reduce(se, e, axis=AX.X, op=ALU.add)
    rs = sm.tile([1, 1], F32, tag="sm_rs")
    nc.vector.reciprocal(rs, se)
    out = sm.tile([1, n], F32, tag="sm_out")
    nc.vector.tensor_scalar(out, e, rs, None, op0=ALU.mult)
    return out


def _col_to_row(nc, tp, sm, col_ap, ident, tag):
    """[128, 1] column (any partitions) -> [1, 128] row on partition 0."""
    ps = tp.tile([128, 256], F32, tag="tp", name="tpt")[:, 0:128]
    nc.tensor.transpose(ps[0:1, :], col_ap, ident)
    row = sm.tile([1, 128], F32, tag=tag)
    nc.scalar.copy(row, ps[0:1, 0:128])
    return row


def _row_to_col(nc, tp, sm, row_ap, one1, tag, scale=1.0):
    """[1, 128] row on partition 0 -> [128, 1] column, via PE transpose."""
    ps = tp.tile([128, 256], F32, tag="tp", name="tpt")[:, 0:128]
    nc.tensor.transpose(ps[:, 0:1], row_ap, one1)
    col = sm.tile([128, 1], F32, tag=tag)
    nc.scalar.activation(col, ps[:, 0:1], AF.Copy, bias=0.0, scale=scale)
    return col


def _dup_row(nc, sm, half_ap, tag):
    """[1, 64] -> [1, 128] duplicated halves."""
    row = sm.tile([1, 128], F32, tag=tag)
    nc.vector.tensor_copy(row[:, 0:64], half_ap)
    nc.vector.tensor_copy(row[:, 64:128], half_ap)
    return row


def _bcast_col_bf16(nc, tp, sm, row128_ap, one1, dst_ap):
    """[1,128] row -> [128, 64] bf16 replicated column block (dst_ap)."""
    ps = tp.tile([128, 256], F32, tag="tp", name="tpt")[:, 0:128]
    nc.tensor.transpose(ps[:, 0:1], row128_ap, one1)
    nc.vector.tensor_copy(dst_ap, ps[:, 0:1].broadcast_to((128, 64)))


def _chan_stats(nc, tp, sm, ident, ssum, ssq, pfx):
    """Per-channel mean/var from per-partition sums.
    ssum/ssq: [128, 1] per-(c,parity) sums of x and x^2 (8192 elems each).
    Returns (mu [1,64], var [1,64]) on partition 0."""
    sr = _col_to_row(nc, tp, sm, ssum, ident, pfx + "sr")
    qr = _col_to_row(nc, tp, sm, ssq, ident, pfx + "qr")
    mu = sm.tile([1, 64], F32, tag=pfx + "mu")
    nc.vector.tensor_tensor(mu, sr[:, 0:64], sr[:, 64:128], op=ALU.add)
    nc.vector.tensor_scalar(mu, mu, 1.0 / 16384.0, None, op0=ALU.mult)
    ex2 = sm.tile([1, 64], F32, tag=pfx + "ex2")
    nc.vector.tensor_tensor(ex2, qr[:, 0:64], qr[:, 64:128], op=ALU.add)
    nc.vector.tensor_scalar(ex2, ex2, 1.0 / 16384.0, None, op0=ALU.mult)
    mq = sm.tile([1, 64], F32, tag=pfx + "mq")
    nc.vector.tensor_tensor(mq, mu, mu, op=ALU.mult)
    var = sm.tile([1, 64], F32, tag=pfx + "var")
    nc.vector.tensor_tensor(var, ex2, mq, op=ALU.subtract)
    return mu, var


def build_kernel(nc: bass.Bass, tc: tile.TileContext, ctx):
    x = nc.dram_tensor("x", [G * C, H, W], F32, kind="ExternalInput").ap()
    w1 = nc.dram_tensor("w1", [G, C, C], F32, kind="ExternalInput").ap()
    b1 = nc.dram_tensor("b1", [G, C], F32, kind="ExternalInput").ap()
    w3 = nc.dram_tensor("w3", [G, C, C, 3, 3], F32, kind="ExternalInput").ap()
    b3 = nc.dram_tensor("b3", [G, C], F32, kind="ExternalInput").ap()
    gnw = nc.dram_tensor("gnw", [G, C], F32, kind="ExternalInput").ap()
    gnb = nc.dram_tensor("gnb", [G, C], F32, kind="ExternalInput").ap()
    y = nc.dram_tensor("y", [G * C, H, W], F32, kind="ExternalOutput").ap()

    big = ctx.enter_context(tc.tile_pool(name="big", bufs=1))
    wp = ctx.enter_context(tc.tile_pool(name="wp", bufs=1))
    sm = ctx.enter_context(tc.tile_pool(name="sm", bufs=2))
    sgw = ctx.enter_context(tc.tile_pool(name="sgw", bufs=4))
    pp = ctx.enter_context(tc.tile_pool(name="pp", bufs=5, space="PSUM"))
    pw = ctx.enter_context(tc.tile_pool(name="pw", bufs=2, space="PSUM"))
    tp = ctx.enter_context(tc.tile_pool(name="tp", bufs=1, space="PSUM"))

    # ---------------- persistent big tiles ----------------
    t_feat = big.tile([128, 66, 130], F32)    # fp32 feature (stats/output)
    t_featb = big.tile([128, 66, 130], BF16)  # bf16 shadow (conv input)
    t_gx = big.tile([128, 64, 128], F32)      # staging + gated
    t_sgb = big.tile([128, 64, 128], BF16)    # sigmoid(GN(feat))
    t_x1 = big.tile([128, 64, 128], BF16)     # x1 = GN(gated)
    t_x2 = big.tile([128, 64, 128], BF16)     # conv3x3 output

    # ---------------- constants ----------------
    ident = wp.tile([128, 128], F32)
    make_identity(nc, ident)
    ones128 = wp.tile([1, 128], F32)
    nc.vector.memset(ones128, 1.0)
    one1 = ones128[0:1, 0:1]
    epst = wp.tile([1, 1], F32)
    nc.vector.memset(epst, EPS)

    # zero halo rows and pad cols of the bf16 conv input
    nc.vector.memset(t_featb[:, 0, :], 0.0)
    nc.vector.memset(t_featb[:, 65, :], 0.0)
    nc.vector.memset(t_featb[:, :, 0:1], 0.0)
    nc.vector.memset(t_featb[:, :, 129:130], 0.0)

    # ---------------- prepack params ----------------
    w1raw = wp.tile([64, G, 64], F32)
    nc.sync.dma_start(out=w1raw, in_=w1.rearrange("g o c -> o g c"))
    b1r = wp.tile([1, G, 64], F32)
    nc.sync.dma_start(out=b1r, in_=b1.rearrange("g c -> (g c)").unsqueeze(0))
    b3r = wp.tile([1, G, 64], F32)
    nc.sync.dma_start(out=b3r, in_=b3.rearrange("g c -> (g c)").unsqueeze(0))
    gwr = wp.tile([1, G, 64], F32)
    nc.sync.dma_start(out=gwr, in_=gnw.rearrange("g c -> (g c)").unsqueeze(0))
    gbr = wp.tile([1, G, 64], F32)
    nc.sync.dma_start(out=gbr, in_=gnb.rearrange("g c -> (g c)").unsqueeze(0))

    # transposed w1 (lhsT [c, o]), prescaled by 1/128 (pool means), bf16
    w1s = wp.tile([64, G, 64], BF16)
    # conv taps, stacked pairs and singles, bf16
    wstk = wp.tile([128, G, 2, 3, 64], BF16)
    wsgl = wp.tile([128, G, 3, 64], BF16)
    # per-group vectors
    b1v = wp.tile([64, G], F32)          # conv1x1 bias per o
    v11r = wp.tile([128, G, 64], BF16)   # softmax(gnb) replicated lhsT
    x11kr = wp.tile([128, G, 64], BF16)  # softmax(gnb)*sigmoid(gnb) lhsT
    krws = wp.tile([1, G, 64], F32)      # sigmoid(gnb) rows
    cb3r = wp.tile([128, G], F32)        # sum(x11 * b3) replicated column

    tc.strict_bb_all_engine_barrier()

    for g in range(G):
        pt = tp.tile([128, 256], F32, tag="tp", name="tpt")[:, 0:128]
        nc.tensor.transpose(pt[0:64, 0:64], w1raw[:, g, :], ident[0:64, 0:64])
        nc.scalar.activation(
            w1s[:, g, :], pt[0:64, 0:64], AF.Copy, bias=0.0, scale=1.0 / 128.0
        )
        w3raw = sm.tile([64, 64, 9], F32, tag="w3raw")
        nc.sync.dma_start(
            out=w3raw, in_=w3[g].rearrange("o c kh kw -> o c (kh kw)")
        )
        # conv taps: tap index t = ky*3 + kx  (ky = dy+1, kx = dx+1)
        # transpose each tap to [c, o], stage in bf16, then 6 grouped DMAs:
        # ky=1 -> stkE[0:64] + stkO[64:128]; ky=2 -> stkE[64:128] + sgl[0:64];
        # ky=0 -> stkO[0:64] + sgl[64:128]
        stage = sm.tile([64, 9, 64], BF16, tag="tapstage")
        for tapidx in range(9):
            src = w3raw[:, :, tapidx]  # [64(o), 64(c)] strided
            ptt = tp.tile([128, 256], F32, tag="tp", name="tpt")[:, 0:128]
            pslice = ptt[0:64, 0:64]
            nc.tensor.transpose(pslice, src, ident[0:64, 0:64])
            nc.scalar.copy(stage[:, tapidx, :], pslice)
        nc.sync.dma_start(out=wstk[0:64, g, 0, :, :], in_=stage[:, 3:6, :])
        nc.sync.dma_start(out=wstk[64:128, g, 1, :, :], in_=stage[:, 3:6, :])
        nc.sync.dma_start(out=wstk[64:128, g, 0, :, :], in_=stage[:, 6:9, :])
        nc.sync.dma_start(out=wsgl[0:64, g, :, :], in_=stage[:, 6:9, :])
        nc.sync.dma_start(out=wstk[0:64, g, 1, :, :], in_=stage[:, 0:3, :])
        nc.sync.dma_start(out=wsgl[64:128, g, :, :], in_=stage[:, 0:3, :])
        # b1 column
        ptb = tp.tile([128, 256], F32, tag="tp", name="tpt")[:, 0:128]
        nc.tensor.transpose(ptb[0:64, 0:1], b1r[:, g, :], one1)
        nc.scalar.copy(b1v[:, g : g + 1], ptb[0:64, 0:1])
        # x11 = softmax(gnb[g]); k = sigmoid(gnb[g])
        x11 = _sigmoid_softmax(nc, sm, gbr[:, g, :], 64)
        x11d = _dup_row(nc, sm, x11, "x11d")
        _bcast_col_bf16(nc, tp, sm, x11d, one1, v11r[:, g, :])
        krow = sm.tile([1, 64], F32, tag="krow")
        nc.scalar.activation(krow, gbr[:, g, :], AF.Sigmoid)
        nc.vector.tensor_copy(krws[:, g, :], krow)
        # x11k = x11 * k, replicated bf16 lhsT
        xkrow = sm.tile([1, 64], F32, tag="xkrow")
        nc.vector.tensor_tensor(xkrow, x11, krow, op=ALU.mult)
        xkd = _dup_row(nc, sm, xkrow, "xkd")
        _bcast_col_bf16(nc, tp, sm, xkd, one1, x11kr[:, g, :])
        # cb3 = sum(x11 * b3), replicated into a [128,1] column
        xb = sm.tile([1, 64], F32, tag="xb")
        nc.vector.tensor_tensor(xb, x11, b3r[:, g, :], op=ALU.mult)
        cb3s = sm.tile([1, 1], F32, tag="cb3s")
        nc.vector.tensor_reduce(cb3s, xb, axis=AX.X, op=ALU.add)
        cb3d = sm.tile([1, 128], F32, tag="cb3d")
        nc.vector.tensor_scalar(cb3d, ones128, cb3s, None, op0=ALU.mult)
        ptc = tp.tile([128, 256], F32, tag="tp", name="tpt")[:, 0:128]
        nc.tensor.transpose(ptc[:, 0:1], cb3d, one1)
        nc.scalar.copy(cb3r[:, g : g + 1], ptc[:, 0:1])

    tc.strict_bb_all_engine_barrier()

    # ---------------- input DMA ----------------
    def dma_in(g, dst_even, dst_odd):
        gc0 = g * C
        nc.sync.dma_start(out=dst_even, in_=x[gc0 : gc0 + 64, 0:128:2, :])
        nc.sync.dma_start(out=dst_odd, in_=x[gc0 : gc0 + 64, 1:128:2, :])

    dma_in(0, t_feat[0:64, 1:65, 1:129], t_feat[64:128, 1:65, 1:129])

    feat_re = t_feat[:, 1:65, 1:129]    # real region [128, 64, 128]
    featb_re = t_featb[:, 1:65, 1:129]

    # ================= group loop =================
    # For g>0, the cascade add (feat += x_g) and the bf16 shadow refresh
    # are done chunk-wise inside the previous group's phase C, so each
    # group starts with feat/featb already up to date.
    for g in range(G):
        if g == 0:
            nc.scalar.copy(featb_re, feat_re)

        # ---- pooled sums (from bf16 shadow) ----
        xh = sm.tile([128, 64], BF16, tag="xh")     # row sums (over w)
        with nc.allow_low_precision(reason="bf16 pooled sums, fp32 accum"):
            nc.vector.tensor_reduce(xh, featb_re, axis=AX.X, op=ALU.add)
        fsum = sm.tile([128, 1], F32, tag="fsum")
        nc.vector.tensor_reduce(fsum, xh, axis=AX.X, op=ALU.add)
        fsq = sm.tile([128, 1], F32, tag="fsq")
        # dummy elementwise output goes to t_x2 (rewritten later by evicts)
        nc.vector.scalar_tensor_tensor(
            out=t_x2[:],
            in0=featb_re,
            scalar=1.0,
            in1=featb_re,
            op0=ALU.mult,
            op1=ALU.mult,
            accum_out=fsq,
        )
        xw = sm.tile([128, 128], BF16, tag="xw")    # col sums (over rows j)
        with nc.allow_low_precision(reason="bf16 pooled sums, fp32 accum"):
            nc.vector.tensor_reduce(
                xw, featb_re.rearrange("p j w -> p w j"), axis=AX.X, op=ALU.add
            )

        # ---- feat stats -> rstd/-mu*rstd columns ----
        muf, varf = _chan_stats(nc, tp, sm, ident, fsum, fsq, "f")
        rfr = sm.tile([1, 64], F32, tag="rfr")
        nc.scalar.activation(rfr, varf, AF.Sqrt, bias=epst, scale=1.0)
        nc.vector.reciprocal(rfr, rfr)
        srow = _dup_row(nc, sm, rfr, "srow")
        nmf = sm.tile([1, 64], F32, tag="nmf")
        nc.vector.tensor_tensor(nmf, muf, rfr, op=ALU.mult)
        brow = sm.tile([1, 128], F32, tag="brow")
        nc.vector.tensor_scalar(brow[:, 0:64], nmf, -1.0, None, op0=ALU.mult)
        nc.vector.tensor_scalar(brow[:, 64:128], nmf, -1.0, None, op0=ALU.mult)
        rfv = _row_to_col(nc, tp, sm, srow, one1, "rfv")
        bfv = _row_to_col(nc, tp, sm, brow, one1, "bfv")

        # ---- Sg = sigmoid(GN(feat)) in bf16, with per-channel sums ----
        sSg = sm.tile([128, 1], F32, tag="sSg")
        nc.scalar.activation(
            t_sgb[:], feat_re, AF.Sigmoid, bias=bfv, scale=rfv, accum_out=sSg
        )

        # ---- conv1x1 over pooled stats: hw = w1 @ [xh; xw] / 128 + b1 ----
        cat = sm.tile([64, 256], BF16, tag="cat")
        nc.sync.dma_start(
            out=cat[:, 0:128].rearrange("p (h two) -> p h two", two=2)[:, :, 0:1],
            in_=xh[0:64, :].unsqueeze(2),
        )
        nc.sync.dma_start(
            out=cat[:, 0:128].rearrange("p (h two) -> p h two", two=2)[:, :, 1:2],
            in_=xh[64:128, :].unsqueeze(2),
        )
        xwhi = sm.tile([64, 128], BF16, tag="xwhi")
        nc.sync.dma_start(out=xwhi, in_=xw[64:128, :])
        with nc.allow_low_precision(reason="bf16 pooled sums"):
            nc.vector.tensor_tensor(
                cat[:, 128:256], xw[0:64, :], xwhi, op=ALU.add
            )
        phw = tp.tile([128, 256], F32, tag="tp", name="tpt")[0:64, :]
        nc.tensor.matmul(phw, w1s[:, g, :], cat, start=True, stop=True)
        sighw = sm.tile([64, 256], F32, tag="sighw")
        nc.scalar.activation(
            sighw, phw, AF.Sigmoid, bias=b1v[:, g : g + 1], scale=1.0
        )
        sh_eo = sm.tile([128, 64], F32, tag="sh_eo")
        nc.sync.dma_start(
            out=sh_eo[0:64, :],
            in_=sighw[:, 0:128].rearrange("p (h two) -> p h two", two=2)[:, :, 0],
        )
        nc.sync.dma_start(
            out=sh_eo[64:128, :],
            in_=sighw[:, 0:128].rearrange("p (h two) -> p h two", two=2)[:, :, 1],
        )
        sw_eo = sm.tile([128, 128], F32, tag="sw_eo")
        nc.sync.dma_start(out=sw_eo[0:64, :], in_=sighw[:, 128:256])
        nc.sync.dma_start(out=sw_eo[64:128, :], in_=sighw[:, 128:256])

        # ---- gated = feat * sig(xh) * sig(xw) -> t_gx; x1 = GN(gated) ----
        sh_b = sh_eo.unsqueeze(2).broadcast_to((128, 64, 128))
        sw_b = sw_eo.unsqueeze(1).broadcast_to((128, 64, 128))
        nc.vector.tensor_tensor(
            t_gx[:, 0:RSPL, :], feat_re[:, 0:RSPL, :], sh_b[:, 0:RSPL, :],
            op=ALU.mult,
        )
        nc.gpsimd.tensor_tensor(
            t_gx[:, RSPL:64, :], feat_re[:, RSPL:64, :], sh_b[:, RSPL:64, :],
            op=ALU.mult,
        )
        sgsum = sm.tile([128, 1], F32, tag="sgsum")
        nc.vector.scalar_tensor_tensor(
            out=t_gx[:],
            in0=t_gx[:],
            scalar=1.0,
            in1=sw_b,
            op0=ALU.mult,
            op1=ALU.mult,
            accum_out=sgsum,
        )
        sgsq = sm.tile([128, 1], F32, tag="sgsq")
        # dummy output goes to t_x1 (fully rewritten by the x1 affine next)
        nc.scalar.activation(t_x1[:], t_gx[:], AF.Square, accum_out=sgsq)
        # gated channel stats -> x1 affine
        mug, varg = _chan_stats(nc, tp, sm, ident, sgsum, sgsq, "g")
        rgr = sm.tile([1, 64], F32, tag="rgr")
        nc.scalar.activation(rgr, varg, AF.Sqrt, bias=epst, scale=1.0)
        nc.vector.reciprocal(rgr, rgr)
        s1 = sm.tile([1, 64], F32, tag="s1")
        nc.vector.tensor_tensor(s1, gwr[:, g, :], rgr, op=ALU.mult)
        nmg = sm.tile([1, 64], F32, tag="nmg")
        nc.vector.tensor_tensor(nmg, mug, s1, op=ALU.mult)
        bx1 = sm.tile([1, 64], F32, tag="bx1")
        nc.vector.scalar_tensor_tensor(
            bx1, nmg, -1.0, gbr[:, g, :], op0=ALU.mult, op1=ALU.add
        )
        s1row = _dup_row(nc, sm, s1, "s1row")
        b1row = _dup_row(nc, sm, bx1, "b1row")
        s1v = _row_to_col(nc, tp, sm, s1row, one1, "s1v")
        b1xv = _row_to_col(nc, tp, sm, b1row, one1, "b1xv")
        # x1 affine on DVE (keeps the Scalar engine free for evictions)
        nc.vector.scalar_tensor_tensor(
            out=t_x1[:],
            in0=t_gx[:],
            scalar=s1v,
            in1=b1xv.unsqueeze(2).broadcast_to((128, 64, 128)),
            op0=ALU.mult,
            op1=ALU.add,
        )
        # stage x_{g+1} into t_gx (now dead) for the phase-C cascade add
        if g + 1 < G:
            dma_in(g + 1, t_gx[0:64, :, :], t_gx[64:128, :, :])

        # ---- conv3x3 (bf16) + x2 eviction on Scalar engine ----
        for par in range(2):
            pbase = 64 * par
            for ci in range(NCHUNK):
                jb = 4 * ci
                pc = pp.tile([128, 512], F32, tag="pconv")
                out_ap = pc[pbase : pbase + 64, :]
                first = True
                for dx in range(3):
                    # stacked pair (K=128)
                    nc.tensor.matmul(
                        out_ap,
                        wstk[:, g, par, dx, :],
                        t_featb[:, 1 + jb : 5 + jb, dx : dx + 128],
                        start=first,
                        stop=False,
                        tile_position=(0, pbase),
                    )
                    first = False
                for dx in range(3):
                    # single tap (K=64)
                    if par == 0:
                        rhs = t_featb[64:128, jb : 4 + jb, dx : dx + 128]
                        lhs = wsgl[64:128, g, dx, :]
                        tpos = (64, 0)
                    else:
                        rhs = t_featb[0:64, 2 + jb : 6 + jb, dx : dx + 128]
                        lhs = wsgl[0:64, g, dx, :]
                        tpos = (0, 64)
                    nc.tensor.matmul(
                        out_ap,
                        lhs,
                        rhs,
                        start=False,
                        stop=(dx == 2),
                        tile_position=tpos,
                    )
                # evict conv PSUM -> bf16 x2
                nc.scalar.activation(
                    t_x2[pbase : pbase + 64, jb : jb + 4, :],
                    pc[pbase : pbase + 64, :].rearrange("p (a b) -> p a b", a=4),
                    AF.Copy,
                )

        # ---- x21 from analytic sums: mean(x2) needs no evictions ----
        # S_t (valid-region input sum per conv tap t) = F - edge_row -
        # edge_col + corner, built from the pooled sums already computed.
        # mean(x12)*HW = sum_t w_t @ S_t + k*sum(Sg); x21 = softmax(...).
        Fc = sm.tile([64, 1], F32, tag="Fc")
        nc.vector.tensor_reduce(Fc, cat[:, 128:256], axis=AX.X, op=ALU.add)
        edg = sm.tile([64, 4], F32, tag="edg")  # R+, R-, C+, C-
        nc.vector.tensor_copy(edg[:, 0:1], cat[0:64, 0:1])
        nc.vector.tensor_copy(edg[:, 1:2], cat[0:64, 127:128])
        nc.vector.tensor_copy(edg[:, 2:3], cat[0:64, 128:129])
        nc.vector.tensor_copy(edg[:, 3:4], cat[0:64, 255:256])
        corn = sm.tile([64, 4], F32, tag="corn")  # x00, x0W, xH0, xHW
        nc.vector.tensor_copy(corn[:, 0:1], t_feat[0:64, 1, 1:2])
        nc.vector.tensor_copy(corn[:, 1:2], t_feat[0:64, 1, 128:129])
        nc.sync.dma_start(out=corn[:, 2:3], in_=t_feat[64:128, 64, 1:2])
        nc.sync.dma_start(out=corn[:, 3:4], in_=t_feat[64:128, 64, 128:129])
        S9 = sm.tile([64, 9], F32, tag="S9")
        nc.vector.tensor_copy(S9, Fc.broadcast_to((64, 9)))
        # dy=-1 rows (t 0..2) lose row H-1 (R-); dy=+1 rows (t 6..8) lose row 0
        nc.vector.tensor_scalar(
            S9[:, 0:3], S9[:, 0:3], edg[:, 1:2], None, op0=ALU.subtract
        )
        nc.vector.tensor_scalar(
            S9[:, 6:9], S9[:, 6:9], edg[:, 0:1], None, op0=ALU.subtract
        )
        # dx=-1 cols (t 0,3,6) lose col W-1 (C-); dx=+1 cols (t 2,5,8) lose col 0
        nc.vector.tensor_scalar(
            S9[:, 0:9:3], S9[:, 0:9:3], edg[:, 3:4], None, op0=ALU.subtract
        )
        nc.vector.tensor_scalar(
            S9[:, 2:9:3], S9[:, 2:9:3], edg[:, 2:3], None, op0=ALU.subtract
        )
        # corner add-back for the four diagonal taps
        nc.vector.tensor_tensor(S9[:, 0:1], S9[:, 0:1], corn[:, 3:4], op=ALU.add)
        nc.vector.tensor_tensor(S9[:, 2:3], S9[:, 2:3], corn[:, 2:3], op=ALU.add)
        nc.vector.tensor_tensor(S9[:, 6:7], S9[:, 6:7], corn[:, 1:2], op=ALU.add)
        nc.vector.tensor_tensor(S9[:, 8:9], S9[:, 8:9], corn[:, 0:1], op=ALU.add)
        S9b = sm.tile([64, 9], BF16, tag="S9b")
        nc.vector.tensor_copy(S9b, S9)
        # sum_t S_t^T @ w_t -> [1, 64] row via 9 accumulating matmuls
        pm = tp.tile([128, 256], F32, tag="tp", name="tpt")[:, 0:128]
        taps = [
            wstk[0:64, g, 1, 0, :], wstk[0:64, g, 1, 1, :], wstk[0:64, g, 1, 2, :],
            wstk[0:64, g, 0, 0, :], wstk[0:64, g, 0, 1, :], wstk[0:64, g, 0, 2, :],
            wsgl[0:64, g, 0, :], wsgl[0:64, g, 1, :], wsgl[0:64, g, 2, :],
        ]
        for t in range(9):
            nc.tensor.matmul(
                pm[0:1, 0:64],
                S9b[:, t : t + 1],
                taps[t],
                start=(t == 0),
                stop=(t == 8),
                tile_position=(0, 0),
            )
        pmr = sm.tile([1, 64], F32, tag="pmr")
        nc.scalar.copy(pmr, pm[0:1, 0:64])
        sSgrow = _col_to_row(nc, tp, sm, sSg, ident, "sSgrow")
        sSgf = sm.tile([1, 64], F32, tag="sSgf")
        nc.vector.tensor_tensor(
            sSgf, sSgrow[:, 0:64], sSgrow[:, 64:128], op=ALU.add
        )
        nc.vector.tensor_tensor(sSgf, sSgf, krws[:, g, :], op=ALU.mult)
        nc.vector.tensor_tensor(sSgf, sSgf, pmr, op=ALU.add)
        x21in = sm.tile([1, 64], F32, tag="x21in")
        nc.vector.scalar_tensor_tensor(
            x21in, sSgf, 1.0 / 16384.0, b3r[:, g, :], op0=ALU.mult, op1=ALU.add
        )
        x21 = _sigmoid_softmax(nc, sm, x21in, 64)
        x21d = _dup_row(nc, sm, x21, "x21d")
        v21r = sm.tile([128, 64], BF16, tag="v21r")
        _bcast_col_bf16(nc, tp, sm, x21d, one1, v21r[:, :])

        # ---- phase C: weights = x11@x2 + (x11*k)@Sg + x21@x1;
        #      out = feat*sig(w); then per-chunk y DMA, cascade add and
        #      bf16 shadow refresh for the next group (fully pipelined) ----
        gc0 = g * C
        for par in range(2):
            pbase = 64 * par
            psl = slice(pbase, pbase + 64)
            for cj in range(8):
                jb = 8 * cj
                rsl = slice(jb, jb + 8)
                sw_c = sgw.tile([128, 1024], F32, tag="sw_c")
                for h in range(2):
                    osl = slice(512 * h, 512 * h + 512)
                    hsl = slice(jb + 4 * h, jb + 4 * h + 4)
                    pwt = pw.tile([128, 512], F32, tag="pwts")
                    nc.tensor.matmul(
                        pwt[psl, :],
                        v11r[psl, g, :],
                        t_x2[psl, hsl, :],
                        start=True,
                        stop=False,
                        tile_position=(pbase, pbase),
                    )
                    nc.tensor.matmul(
                        pwt[psl, :],
                        x11kr[psl, g, :],
                        t_sgb[psl, hsl, :],
                        start=False,
                        stop=False,
                        tile_position=(pbase, pbase),
                    )
                    nc.tensor.matmul(
                        pwt[psl, :],
                        v21r[psl, :],
                        t_x1[psl, hsl, :],
                        start=False,
                        stop=True,
                        tile_position=(pbase, pbase),
                    )
                    nc.scalar.activation(
                        sw_c[psl, osl], pwt[psl, :], AF.Sigmoid,
                        bias=cb3r[psl, g : g + 1], scale=1.0,
                    )
                fsl = (psl, slice(1 + jb, 9 + jb), slice(1, 129))
                eng = nc.vector if cj < 5 else nc.gpsimd
                eng.tensor_tensor(
                    t_feat[fsl],
                    t_feat[fsl],
                    sw_c[psl, :].rearrange("p (a b) -> p a b", a=8),
                    op=ALU.mult,
                )
                # y chunk out
                nc.sync.dma_start(
                    out=y[gc0 : gc0 + 64, 2 * jb + par : 2 * jb + 16 : 2, :],
                    in_=t_feat[fsl],
                )
                if g + 1 < G:
                    # cascade add + bf16 shadow refresh for next group
                    eng2 = nc.vector if cj >= 5 else nc.gpsimd
                    eng2.tensor_tensor(
                        t_feat[fsl], t_feat[fsl], t_gx[psl, rsl, :], op=ALU.add
                    )
                    nc.scalar.copy(t_featb[fsl], t_feat[fsl])

    return nc


_CACHE = {}


def _get_nc(split=True):
    if "nc" not in _CACHE:
        from contextlib import ExitStack

        nc = bacc.Bacc(
            "TRN2", target_bir_lowering=False, debug=False, num_devices=8
        )
        with tile.TileContext(nc) as tc:
            with ExitStack() as ctx:
                build_kernel(nc, tc, ctx)
        nc.compile()
        _CACHE["nc"] = nc
    return _CACHE["nc"]


def kernel(x, w1, b1, w3, b3, gnw, gnb):
    nc = _get_nc()
    from concourse.bass_utils import run_bass_kernel_spmd

    x = np.ascontiguousarray(np.asarray(x, dtype=np.float32))
    params = {
        "w1": np.ascontiguousarray(np.asarray(w1, np.float32)),
        "b1": np.ascontiguousarray(np.asarray(b1, np.float32)),
        "w3": np.ascontiguousarray(np.asarray(w3, np.float32)),
        "b3": np.ascontiguousarray(np.asarray(b3, np.float32)),
        "gnw": np.ascontiguousarray(np.asarray(gnw, np.float32)),
        "gnb": np.ascontiguousarray(np.asarray(gnb, np.float32)),
    }
    in_maps = [dict(params, x=np.ascontiguousarray(x[i])) for i in range(8)]
    res = run_bass_kernel_spmd(nc, in_maps, list(range(8)))
    out = np.stack([res.results[i]["y"] for i in range(8)], axis=0)
    return out


# revision 33
# speedup vs baseline: 1.2998x; 1.2998x over previous
"""Trainium2 Bass kernel for nn_CascadedSpatialCrossAttention.

Sharding: data-parallel over batch. B=8 batch elements -> 8 NeuronCores,
one batch element per core. Params are replicated. No collectives.

Per-core layout: an image tensor (64ch, 128, 128) is stored with
partition p = c + 64*parity (parity = h % 2), i.e. even rows of channel c
on partition c, odd rows on partition c+64.  This uses all 128 partitions
for elementwise/reduce work and lets conv3x3 taps be K-stacked in pairs
(even+odd source rows share one free-dim offset).

v2: all hot matmuls run in bf16 (4x PE throughput vs fp32):
  - conv3x3 reads a bf16 shadow copy of the feature tile (t_featb)
  - conv PSUM is evicted by the Scalar engine (Copy + accum) to bf16
  - the spatial-weights matmuls use 64-wide replicated lhsT columns so
    their PSUM output is already broadcast across partitions (the old
    ones-matmul replication step is gone)
  - x12 = x2 + k*Sg is never materialized; its two terms enter the
    weights matmul as separate bf16 rhs operands (x11@x2 + (x11*k)@Sg)
Elementwise work is split between Vector (DVE) and GpSimd (Pool) engines.
"""

import sys

sys.path.insert(0, "/opt/trn_rl_repo")

import numpy as np

import concourse.bass as bass
import concourse.bacc as bacc
import concourse.tile as tile
from concourse import mybir
from concourse.masks import make_identity

F32 = mybir.dt.float32
BF16 = mybir.dt.bfloat16
AF = mybir.ActivationFunctionType
ALU = mybir.AluOpType
AX = mybir.AxisListType

G = 4          # groups
C = 64         # channels per group
H = W = 128
J = 64         # row pairs per parity
NCHUNK = 16    # free-dim chunks of 512 (4 rows) per parity
EPS = 1e-5

# row split for DVE/Pool elementwise sharing (Pool is ~2x slower per elem)
RSPL = 42
# chunk split for the final multiply (DVE chunks < CSPL, Pool the rest)
CSPL = 21


def _sigmoid_softmax(nc, sm, vec, n):
    """softmax over vec [1, n] (SBUF, partition 0), using sigmoid-based exp
    (exp(z) = s/(1-s), s = sigmoid(z), z <= 0 after max subtraction)."""
    mx = sm.tile([1, 1], F32, tag="sm_mx")
    nc.vector.tensor_reduce(mx, vec, axis=AX.X, op=ALU.max)
    z = sm.tile([1, n], F32, tag="sm_z")
    nc.vector.tensor_scalar(z, vec, mx, None, op0=ALU.subtract)
    s = sm.tile([1, n], F32, tag="sm_s")
    nc.scalar.activation(s, z, AF.Sigmoid)
    u = sm.tile([1, n], F32, tag="sm_u")
    nc.vector.tensor_scalar(u, s, -1.0, 1.0, op0=ALU.mult, op1=ALU.add)
    r = sm.tile([1, n], F32, tag="sm_r")
    nc.vector.reciprocal(r, u)
    e = sm.tile([1, n], F32, tag="sm_e")
    nc.vector.tensor_tensor(e, s, r, op=ALU.mult)
    se = sm.tile([1, 1], F32, tag="sm_se")
    nc.vector.tensor_reduce(se, e, axis=AX.X, op=ALU.add)
    rs = sm.tile([1, 1], F32, tag="sm_rs")
    nc.vector.reciprocal(rs, se)
    out = sm.tile([1, n], F32, tag="sm_out")
    nc.vector.tensor_scalar(out, e, rs, None, op0=ALU.mult)
    return out


def _col_to_row(nc, tp, sm, col_ap, ident, tag):
    """[128, 1] column (any partitions) -> [1, 128] row on partition 0."""
    ps = tp.tile([128, 256], F32, tag="tp", name="tpt")[:, 0:128]
    nc.tensor.transpose(ps[0:1, :], col_ap, ident)
    row = sm.tile([1, 128], F32, tag=tag)
    nc.scalar.copy(row, ps[0:1, 0:128])
    return row


def _row_to_col(nc, tp, sm, row_ap, one1, tag, scale=1.0):
    """[1, 128] row on partition 0 -> [128, 1] column, via PE transpose."""
    ps = tp.tile([128, 256], F32, tag="tp", name="tpt")[:, 0:128]
    nc.tensor.transpose(ps[:, 0:1], row_ap, one1)
    col = sm.tile([128, 1], F32, tag=tag)
    nc.scalar.activation(col, ps[:, 0:1], AF.Copy, bias=0.0, scale=scale)
    return col


def _dup_row(nc, sm, half_ap, tag):
    """[1, 64] -> [1, 128] duplicated halves."""
    row = sm.tile([1, 128], F32, tag=tag)
    nc.vector.tensor_copy(row[:, 0:64], half_ap)
    nc.vector.tensor_copy(row[:, 64:128], half_ap)
    return row


def _bcast_col_bf16(nc, tp, sm, row128_ap, one1, dst_ap):
    """[1,128] row -> [128, 64] bf16 replicated column block (dst_ap)."""
    ps = tp.tile([128, 256], F32, tag="tp", name="tpt")[:, 0:128]
    nc.tensor.transpose(ps[:, 0:1], row128_ap, one1)
    nc.vector.tensor_copy(dst_ap, ps[:, 0:1].broadcast_to((128, 64)))



def _bcast_diag_bf16(nc, tp, row128_ap, one1, dst_top, dst_bot):
    """[1,128] dup row -> two [64,64] bf16 diag blocks (partition-local)."""
    ps = tp.tile([128, 256], F32, tag="tp", name="tpt")[:, 0:128]
    nc.tensor.transpose(ps[:, 0:1], row128_ap, one1)
    nc.vector.tensor_copy(dst_top, ps[0:64, 0:1].broadcast_to((64, 64)))
    nc.vector.tensor_copy(dst_bot, ps[64:128, 0:1].broadcast_to((64, 64)))

def _chan_stats(nc, tp, sm, ident, ssum, ssq, pfx):
    """Per-channel mean/var from per-partition sums.
    ssum/ssq: [128, 1] per-(c,parity) sums of x and x^2 (8192 elems each).
    Returns (mu [1,64], var [1,64]) on partition 0."""
    sr = _col_to_row(nc, tp, sm, ssum, ident, pfx + "sr")
    qr = _col_to_row(nc, tp, sm, ssq, ident, pfx + "qr")
    mu = sm.tile([1, 64], F32, tag=pfx + "mu")
    nc.vector.tensor_tensor(mu, sr[:, 0:64], sr[:, 64:128], op=ALU.add)
    nc.vector.tensor_scalar(mu, mu, 1.0 / 16384.0, None, op0=ALU.mult)
    ex2 = sm.tile([1, 64], F32, tag=pfx + "ex2")
    nc.vector.tensor_tensor(ex2, qr[:, 0:64], qr[:, 64:128], op=ALU.add)
    nc.vector.tensor_scalar(ex2, ex2, 1.0 / 16384.0, None, op0=ALU.mult)
    mq = sm.tile([1, 64], F32, tag=pfx + "mq")
    nc.vector.tensor_tensor(mq, mu, mu, op=ALU.mult)
    var = sm.tile([1, 64], F32, tag=pfx + "var")
    nc.vector.tensor_tensor(var, ex2, mq, op=ALU.subtract)
    return mu, var


def build_kernel(nc: bass.Bass, tc: tile.TileContext, ctx):
    x = nc.dram_tensor("x", [G * C, H, W], F32, kind="ExternalInput").ap()
    w1 = nc.dram_tensor("w1", [G, C, C], F32, kind="ExternalInput").ap()
    b1 = nc.dram_tensor("b1", [G, C], F32, kind="ExternalInput").ap()
    w3 = nc.dram_tensor("w3", [G, C, C, 3, 3], F32, kind="ExternalInput").ap()
    b3 = nc.dram_tensor("b3", [G, C], F32, kind="ExternalInput").ap()
    gnw = nc.dram_tensor("gnw", [G, C], F32, kind="ExternalInput").ap()
    gnb = nc.dram_tensor("gnb", [G, C], F32, kind="ExternalInput").ap()
    y = nc.dram_tensor("y", [G * C, H, W], F32, kind="ExternalOutput").ap()

    big = ctx.enter_context(tc.tile_pool(name="big", bufs=1))
    wp = ctx.enter_context(tc.tile_pool(name="wp", bufs=1))
    sm = ctx.enter_context(tc.tile_pool(name="sm", bufs=2))
    sgw = ctx.enter_context(tc.tile_pool(name="sgw", bufs=2))
    pp = ctx.enter_context(tc.tile_pool(name="pp", bufs=3, space="PSUM"))
    pw = ctx.enter_context(tc.tile_pool(name="pw", bufs=2, space="PSUM"))
    tp = ctx.enter_context(tc.tile_pool(name="tp", bufs=1, space="PSUM"))

    # ---------------- persistent big tiles ----------------
    t_feat = big.tile([128, 66, 130], F32)    # fp32 feature (stats/output)
    t_featb = big.tile([128, 66, 130], BF16)  # bf16 shadow (conv input)
    t_gx = big.tile([128, 64, 128], F32)      # staging + gated
    t_sgb = big.tile([128, 64, 128], BF16)    # sigmoid(GN(feat))
    t_x1 = big.tile([128, 64, 128], BF16)     # x1 = GN(gated)
    t_x2 = big.tile([128, 64, 128], BF16)     # conv3x3 output
    # partition-swapped bf16 shadow for the leftover conv taps:
    # top half = par1 rows shifted -1 (ky0 taps for even outputs),
    # bottom half = par0 rows shifted +1 (ky2 taps for odd outputs)
    t_featd = big.tile([128, 66, 130], BF16)

    # ---------------- constants ----------------
    ident = wp.tile([128, 128], F32)
    make_identity(nc, ident)
    ones128 = wp.tile([1, 128], F32)
    nc.vector.memset(ones128, 1.0)
    one1 = ones128[0:1, 0:1]
    epst = wp.tile([1, 1], F32)
    nc.vector.memset(epst, EPS)

    # zero halo rows and pad cols of the bf16 conv input
    nc.vector.memset(t_featb[:, 0, :], 0.0)
    nc.vector.memset(t_featb[:, 65, :], 0.0)
    nc.vector.memset(t_featb[:, :, 0:1], 0.0)
    nc.vector.memset(t_featb[:, :, 129:130], 0.0)

    # ---------------- prepack params ----------------
    w1raw = wp.tile([64, G, 64], F32)
    nc.sync.dma_start(out=w1raw, in_=w1.rearrange("g o c -> o g c"))
    b1r = wp.tile([1, G, 64], F32)
    nc.sync.dma_start(out=b1r, in_=b1.rearrange("g c -> (g c)").unsqueeze(0))
    b3r = wp.tile([1, G, 64], F32)
    nc.sync.dma_start(out=b3r, in_=b3.rearrange("g c -> (g c)").unsqueeze(0))
    gwr = wp.tile([1, G, 64], F32)
    nc.sync.dma_start(out=gwr, in_=gnw.rearrange("g c -> (g c)").unsqueeze(0))
    gbr = wp.tile([1, G, 64], F32)
    nc.sync.dma_start(out=gbr, in_=gnb.rearrange("g c -> (g c)").unsqueeze(0))

    # transposed w1 (lhsT [c, o]), prescaled by 1/128 (pool means), bf16
    w1s = wp.tile([64, G, 64], BF16)
    # conv taps, stacked pairs and singles, bf16
    wstk = wp.tile([128, G, 2, 3, 64], BF16)
    wsgl = wp.tile([128, G, 3, 64], BF16)
    # per-group vectors
    b1v = wp.tile([64, G], F32)          # conv1x1 bias per o
    v11d = wp.tile([128, G, 128], BF16)  # block-diag softmax(gnb) lhsT
    x11kd = wp.tile([128, G, 128], BF16)  # block-diag x11*k lhsT
    wdia = wp.tile([128, G, 3, 128], BF16)  # block-diag ky0/ky2 conv taps
    krws = wp.tile([1, G, 64], F32)      # sigmoid(gnb) rows
    cb3r = wp.tile([128, G], F32)        # sum(x11 * b3) replicated column
    nc.vector.memset(v11d, 0.0)
    nc.vector.memset(x11kd, 0.0)
    nc.vector.memset(wdia, 0.0)

    w3raw = wp.tile([64, 64, 9], F32)
    stage0 = wp.tile([64, 9, 64], BF16)

    tc.strict_bb_all_engine_barrier()

    for g in range(G):
        pt = tp.tile([128, 256], F32, tag="tp", name="tpt")[:, 0:128]
        nc.tensor.transpose(pt[0:64, 0:64], w1raw[:, g, :], ident[0:64, 0:64])
        nc.scalar.activation(
            w1s[:, g, :], pt[0:64, 0:64], AF.Copy, bias=0.0, scale=1.0 / 128.0
        )
        nc.sync.dma_start(
            out=w3raw, in_=w3[g].rearrange("o c kh kw -> o c (kh kw)")
        )
        # conv taps: tap index t = ky*3 + kx  (ky = dy+1, kx = dx+1)
        # transpose each tap to [c, o], stage in bf16, then 6 grouped DMAs:
        # ky=1 -> stkE[0:64] + stkO[64:128]; ky=2 -> stkE[64:128] + sgl[0:64];
        # ky=0 -> stkO[0:64] + sgl[64:128]
        stage = stage0
        for tapidx in range(9):
            src = w3raw[:, :, tapidx]  # [64(o), 64(c)] strided
            ptt = tp.tile([128, 256], F32, tag="tp", name="tpt")[:, 0:128]
            pslice = ptt[0:64, 0:64]
            nc.tensor.transpose(pslice, src, ident[0:64, 0:64])
            nc.scalar.copy(stage[:, tapidx, :], pslice)
        nc.sync.dma_start(out=wstk[0:64, g, 0, :, :], in_=stage[:, 3:6, :])
        nc.sync.dma_start(out=wstk[64:128, g, 1, :, :], in_=stage[:, 3:6, :])
        nc.sync.dma_start(out=wstk[64:128, g, 0, :, :], in_=stage[:, 6:9, :])
        nc.sync.dma_start(out=wsgl[0:64, g, :, :], in_=stage[:, 6:9, :])
        nc.sync.dma_start(out=wstk[0:64, g, 1, :, :], in_=stage[:, 0:3, :])
        nc.sync.dma_start(out=wsgl[64:128, g, :, :], in_=stage[:, 0:3, :])
        # block-diag leftover taps: top-left ky0 (even outs from par1 data),
        # bottom-right ky2 (odd outs from par0 data)
        nc.sync.dma_start(out=wdia[0:64, g, :, 0:64], in_=stage[:, 0:3, :])
        nc.sync.dma_start(out=wdia[64:128, g, :, 64:128], in_=stage[:, 6:9, :])
        # b1 column
        ptb = tp.tile([128, 256], F32, tag="tp", name="tpt")[:, 0:128]
        nc.tensor.transpose(ptb[0:64, 0:1], b1r[:, g, :], one1)
        nc.scalar.copy(b1v[:, g : g + 1], ptb[0:64, 0:1])
        # x11 = softmax(gnb[g]); k = sigmoid(gnb[g])
        x11 = _sigmoid_softmax(nc, sm, gbr[:, g, :], 64)
        x11dd = _dup_row(nc, sm, x11, "x11dd")
        _bcast_diag_bf16(
            nc, tp, x11dd, one1, v11d[0:64, g, 0:64], v11d[64:128, g, 64:128]
        )
        krow = sm.tile([1, 64], F32, tag="krow")
        nc.scalar.activation(krow, gbr[:, g, :], AF.Sigmoid)
        nc.vector.tensor_copy(krws[:, g, :], krow)
        # x11k = x11 * k, replicated bf16 lhsT
        xkrow = sm.tile([1, 64], F32, tag="xkrow")
        nc.vector.tensor_tensor(xkrow, x11, krow, op=ALU.mult)
        xkd = _dup_row(nc, sm, xkrow, "xkd")
        _bcast_diag_bf16(
            nc, tp, xkd, one1, x11kd[0:64, g, 0:64], x11kd[64:128, g, 64:128]
        )
        # cb3 = sum(x11 * b3), replicated into a [128,1] column
        xb = sm.tile([1, 64], F32, tag="xb")
        nc.vector.tensor_tensor(xb, x11, b3r[:, g, :], op=ALU.mult)
        cb3s = sm.tile([1, 1], F32, tag="cb3s")
        nc.vector.tensor_reduce(cb3s, xb, axis=AX.X, op=ALU.add)
        cb3d = sm.tile([1, 128], F32, tag="cb3d")
        nc.vector.tensor_scalar(cb3d, ones128, cb3s, None, op0=ALU.mult)
        ptc = tp.tile([128, 256], F32, tag="tp", name="tpt")[:, 0:128]
        nc.tensor.transpose(ptc[:, 0:1], cb3d, one1)
        nc.scalar.copy(cb3r[:, g : g + 1], ptc[:, 0:1])

    tc.strict_bb_all_engine_barrier()

    # ---------------- input DMA ----------------
    def dma_in(g, dst_even, dst_odd):
        gc0 = g * C
        nc.sync.dma_start(out=dst_even, in_=x[gc0 : gc0 + 64, 0:128:2, :])
        nc.sync.dma_start(out=dst_odd, in_=x[gc0 : gc0 + 64, 1:128:2, :])

    dma_in(0, t_feat[0:64, 1:65, 1:129], t_feat[64:128, 1:65, 1:129])

    feat_re = t_feat[:, 1:65, 1:129]    # real region [128, 64, 128]
    featb_re = t_featb[:, 1:65, 1:129]

    # ================= group loop =================
    # For g>0, the cascade add (feat += x_g) and the bf16 shadow refresh
    # are done chunk-wise inside the previous group's phase C, so each
    # group starts with feat/featb already up to date.
    for g in range(G):
        if g == 0:
            nc.scalar.copy(featb_re, feat_re)
        # partition-swapped shadow: top = par1 rows j-1 (includes zero halo
        # row 0), bottom = par0 rows j+1 (includes zero halo row 65)
        nc.sync.dma_start(
            out=t_featd[0:64, 1:65, :], in_=t_featb[64:128, 0:64, :]
        )
        nc.sync.dma_start(
            out=t_featd[64:128, 1:65, :], in_=t_featb[0:64, 2:66, :]
        )

        # ---- pooled sums (from bf16 shadow) ----
        xh = sm.tile([128, 64], BF16, tag="xh")     # row sums (over w)
        with nc.allow_low_precision(reason="bf16 pooled sums, fp32 accum"):
            nc.vector.tensor_reduce(xh, featb_re, axis=AX.X, op=ALU.add)
        fsum = sm.tile([128, 1], F32, tag="fsum")
        nc.vector.tensor_reduce(fsum, xh, axis=AX.X, op=ALU.add)
        fsq = sm.tile([128, 1], F32, tag="fsq")
        # dummy elementwise output goes to t_x2 (rewritten later by evicts)
        nc.vector.scalar_tensor_tensor(
            out=t_x2[:],
            in0=featb_re,
            scalar=1.0,
            in1=featb_re,
            op0=ALU.mult,
            op1=ALU.mult,
            accum_out=fsq,
        )
        xw = sm.tile([128, 128], BF16, tag="xw")    # col sums (over rows j)
        with nc.allow_low_precision(reason="bf16 pooled sums, fp32 accum"):
            nc.vector.tensor_reduce(
                xw, featb_re.rearrange("p j w -> p w j"), axis=AX.X, op=ALU.add
            )

        # ---- feat stats -> rstd/-mu*rstd columns ----
        muf, varf = _chan_stats(nc, tp, sm, ident, fsum, fsq, "f")
        rfr = sm.tile([1, 64], F32, tag="rfr")
        nc.scalar.activation(rfr, varf, AF.Sqrt, bias=epst, scale=1.0)
        nc.vector.reciprocal(rfr, rfr)
        srow = _dup_row(nc, sm, rfr, "srow")
        nmf = sm.tile([1, 64], F32, tag="nmf")
        nc.vector.tensor_tensor(nmf, muf, rfr, op=ALU.mult)
        brow = sm.tile([1, 128], F32, tag="brow")
        nc.vector.tensor_scalar(brow[:, 0:64], nmf, -1.0, None, op0=ALU.mult)
        nc.vector.tensor_scalar(brow[:, 64:128], nmf, -1.0, None, op0=ALU.mult)
        rfv = _row_to_col(nc, tp, sm, srow, one1, "rfv")
        bfv = _row_to_col(nc, tp, sm, brow, one1, "bfv")

        # ---- Sg = sigmoid(GN(feat)) in bf16, with per-channel sums ----
        sSg = sm.tile([128, 1], F32, tag="sSg")
        nc.scalar.activation(
            t_sgb[:], feat_re, AF.Sigmoid, bias=bfv, scale=rfv, accum_out=sSg
        )

        # ---- conv1x1 over pooled stats: hw = w1 @ [xh; xw] / 128 + b1 ----
        cat = sm.tile([64, 256], BF16, tag="cat")
        nc.sync.dma_start(
            out=cat[:, 0:128].rearrange("p (h two) -> p h two", two=2)[:, :, 0:1],
            in_=xh[0:64, :].unsqueeze(2),
        )
        nc.sync.dma_start(
            out=cat[:, 0:128].rearrange("p (h two) -> p h two", two=2)[:, :, 1:2],
            in_=xh[64:128, :].unsqueeze(2),
        )
        xwhi = sm.tile([64, 128], BF16, tag="xwhi")
        nc.sync.dma_start(out=xwhi, in_=xw[64:128, :])
        with nc.allow_low_precision(reason="bf16 pooled sums"):
            nc.vector.tensor_tensor(
                cat[:, 128:256], xw[0:64, :], xwhi, op=ALU.add
            )
        phw = tp.tile([128, 256], F32, tag="tp", name="tpt")[0:64, :]
        nc.tensor.matmul(phw, w1s[:, g, :], cat, start=True, stop=True)
        sighw = sm.tile([64, 256], F32, tag="sighw")
        nc.scalar.activation(
            sighw, phw, AF.Sigmoid, bias=b1v[:, g : g + 1], scale=1.0
        )
        sh_eo = sm.tile([128, 64], F32, tag="sh_eo")
        nc.sync.dma_start(
            out=sh_eo[0:64, :],
            in_=sighw[:, 0:128].rearrange("p (h two) -> p h two", two=2)[:, :, 0],
        )
        nc.sync.dma_start(
            out=sh_eo[64:128, :],
            in_=sighw[:, 0:128].rearrange("p (h two) -> p h two", two=2)[:, :, 1],
        )
        sw_eo = sm.tile([128, 128], F32, tag="sw_eo")
        nc.sync.dma_start(out=sw_eo[0:64, :], in_=sighw[:, 128:256])
        nc.sync.dma_start(out=sw_eo[64:128, :], in_=sighw[:, 128:256])

        # ---- gated = feat * sig(xh) * sig(xw) -> t_gx; x1 = GN(gated) ----
        sh_b = sh_eo.unsqueeze(2).broadcast_to((128, 64, 128))
        sw_b = sw_eo.unsqueeze(1).broadcast_to((128, 64, 128))
        nc.vector.tensor_tensor(
            t_gx[:, 0:RSPL, :], feat_re[:, 0:RSPL, :], sh_b[:, 0:RSPL, :],
            op=ALU.mult,
        )
        nc.gpsimd.tensor_tensor(
            t_gx[:, RSPL:64, :], feat_re[:, RSPL:64, :], sh_b[:, RSPL:64, :],
            op=ALU.mult,
        )
        sgsum = sm.tile([128, 1], F32, tag="sgsum")
        nc.vector.scalar_tensor_tensor(
            out=t_gx[:],
            in0=t_gx[:],
            scalar=1.0,
            in1=sw_b,
            op0=ALU.mult,
            op1=ALU.mult,
            accum_out=sgsum,
        )
        sgsq = sm.tile([128, 1], F32, tag="sgsq")
        # dummy output goes to t_x1 (fully rewritten by the x1 affine next)
        nc.scalar.activation(t_x1[:], t_gx[:], AF.Square, accum_out=sgsq)
        # gated channel stats -> x1 affine
        mug, varg = _chan_stats(nc, tp, sm, ident, sgsum, sgsq, "g")
        rgr = sm.tile([1, 64], F32, tag="rgr")
        nc.scalar.activation(rgr, varg, AF.Sqrt, bias=epst, scale=1.0)
        nc.vector.reciprocal(rgr, rgr)
        s1 = sm.tile([1, 64], F32, tag="s1")
        nc.vector.tensor_tensor(s1, gwr[:, g, :], rgr, op=ALU.mult)
        nmg = sm.tile([1, 64], F32, tag="nmg")
        nc.vector.tensor_tensor(nmg, mug, s1, op=ALU.mult)
        bx1 = sm.tile([1, 64], F32, tag="bx1")
        nc.vector.scalar_tensor_tensor(
            bx1, nmg, -1.0, gbr[:, g, :], op0=ALU.mult, op1=ALU.add
        )
        s1row = _dup_row(nc, sm, s1, "s1row")
        b1row = _dup_row(nc, sm, bx1, "b1row")
        s1v = _row_to_col(nc, tp, sm, s1row, one1, "s1v")
        b1xv = _row_to_col(nc, tp, sm, b1row, one1, "b1xv")
        # x1 affine on DVE (keeps the Scalar engine free for evictions)
        nc.vector.scalar_tensor_tensor(
            out=t_x1[:],
            in0=t_gx[:],
            scalar=s1v,
            in1=b1xv.unsqueeze(2).broadcast_to((128, 64, 128)),
            op0=ALU.mult,
            op1=ALU.add,
        )
        # stage x_{g+1} into t_gx (now dead) for the phase-C cascade add
        if g + 1 < G:
            dma_in(g + 1, t_gx[0:64, :, :], t_gx[64:128, :, :])

        # ---- conv3x3 (bf16) + x2 eviction on Scalar engine ----
        for ci in range(NCHUNK):
            jb = 4 * ci
            pc = pp.tile([128, 512], F32, tag="pconv")
            for dx in range(3):
                # even outputs: par0 rows j (ky1) + par1 rows j (ky2), K=128
                nc.tensor.matmul(
                    pc[0:64, :],
                    wstk[:, g, 0, dx, :],
                    t_featb[:, 1 + jb : 5 + jb, dx : dx + 128],
                    start=(dx == 0),
                    stop=False,
                    tile_position=(0, 0),
                    skip_group_check=True,
                )
            for dx in range(3):
                # odd outputs: par0 rows j (ky0) + par1 rows j (ky1), K=128
                nc.tensor.matmul(
                    pc[64:128, :],
                    wstk[:, g, 1, dx, :],
                    t_featb[:, 1 + jb : 5 + jb, dx : dx + 128],
                    start=(dx == 0),
                    stop=False,
                    tile_position=(0, 64),
                    skip_group_check=True,
                )
            for dx in range(3):
                # leftover taps, both parities via block-diag lhsT:
                # top: par1 rows j-1 (ky0) -> even outs;
                # bottom: par0 rows j+1 (ky2) -> odd outs
                nc.tensor.matmul(
                    pc[:, :],
                    wdia[:, g, dx, :],
                    t_featd[:, 1 + jb : 5 + jb, dx : dx + 128],
                    start=False,
                    stop=(dx == 2),
                    tile_position=(0, 0),
                    skip_group_check=True,
                )
            # evict conv PSUM -> bf16 x2 (both parities at once)
            nc.scalar.activation(
                t_x2[:, jb : jb + 4, :],
                pc[:, :].rearrange("p (a b) -> p a b", a=4),
                AF.Copy,
            )

        # ---- x21 from analytic sums: mean(x2) needs no evictions ----
        # S_t (valid-region input sum per conv tap t) = F - edge_row -
        # edge_col + corner, built from the pooled sums already computed.
        # mean(x12)*HW = sum_t w_t @ S_t + k*sum(Sg); x21 = softmax(...).
        Fc = sm.tile([64, 1], F32, tag="Fc")
        nc.vector.tensor_reduce(Fc, cat[:, 128:256], axis=AX.X, op=ALU.add)
        edg = sm.tile([64, 4], F32, tag="edg")  # R+, R-, C+, C-
        nc.vector.tensor_copy(edg[:, 0:1], cat[0:64, 0:1])
        nc.vector.tensor_copy(edg[:, 1:2], cat[0:64, 127:128])
        nc.vector.tensor_copy(edg[:, 2:3], cat[0:64, 128:129])
        nc.vector.tensor_copy(edg[:, 3:4], cat[0:64, 255:256])
        corn = sm.tile([64, 4], F32, tag="corn")  # x00, x0W, xH0, xHW
        nc.vector.tensor_copy(corn[:, 0:1], t_feat[0:64, 1, 1:2])
        nc.vector.tensor_copy(corn[:, 1:2], t_feat[0:64, 1, 128:129])
        nc.sync.dma_start(out=corn[:, 2:3], in_=t_feat[64:128, 64, 1:2])
        nc.sync.dma_start(out=corn[:, 3:4], in_=t_feat[64:128, 64, 128:129])
        S9 = sm.tile([64, 9], F32, tag="S9")
        nc.vector.tensor_copy(S9, Fc.broadcast_to((64, 9)))
        # dy=-1 rows (t 0..2) lose row H-1 (R-); dy=+1 rows (t 6..8) lose row 0
        nc.vector.tensor_scalar(
            S9[:, 0:3], S9[:, 0:3], edg[:, 1:2], None, op0=ALU.subtract
        )
        nc.vector.tensor_scalar(
            S9[:, 6:9], S9[:, 6:9], edg[:, 0:1], None, op0=ALU.subtract
        )
        # dx=-1 cols (t 0,3,6) lose col W-1 (C-); dx=+1 cols (t 2,5,8) lose col 0
        nc.vector.tensor_scalar(
            S9[:, 0:9:3], S9[:, 0:9:3], edg[:, 3:4], None, op0=ALU.subtract
        )
        nc.vector.tensor_scalar(
            S9[:, 2:9:3], S9[:, 2:9:3], edg[:, 2:3], None, op0=ALU.subtract
        )
        # corner add-back for the four diagonal taps
        nc.vector.tensor_tensor(S9[:, 0:1], S9[:, 0:1], corn[:, 3:4], op=ALU.add)
        nc.vector.tensor_tensor(S9[:, 2:3], S9[:, 2:3], corn[:, 2:3], op=ALU.add)
        nc.vector.tensor_tensor(S9[:, 6:7], S9[:, 6:7], corn[:, 1:2], op=ALU.add)
        nc.vector.tensor_tensor(S9[:, 8:9], S9[:, 8:9], corn[:, 0:1], op=ALU.add)
        S9b = sm.tile([64, 9], BF16, tag="S9b")
        nc.vector.tensor_copy(S9b, S9)
        # sum_t S_t^T @ w_t -> [1, 64] row via 9 accumulating matmuls
        pm = tp.tile([128, 256], F32, tag="tp", name="tpt")[:, 0:128]
        taps = [
            wstk[0:64, g, 1, 0, :], wstk[0:64, g, 1, 1, :], wstk[0:64, g, 1, 2, :],
            wstk[0:64, g, 0, 0, :], wstk[0:64, g, 0, 1, :], wstk[0:64, g, 0, 2, :],
            wsgl[0:64, g, 0, :], wsgl[0:64, g, 1, :], wsgl[0:64, g, 2, :],
        ]
        for t in range(9):
            nc.tensor.matmul(
                pm[0:1, 0:64],
                S9b[:, t : t + 1],
                taps[t],
                start=(t == 0),
                stop=(t == 8),
                tile_position=(0, 0),
            )
        pmr = sm.tile([1, 64], F32, tag="pmr")
        nc.scalar.copy(pmr, pm[0:1, 0:64])
        sSgrow = _col_to_row(nc, tp, sm, sSg, ident, "sSgrow")
        sSgf = sm.tile([1, 64], F32, tag="sSgf")
        nc.vector.tensor_tensor(
            sSgf, sSgrow[:, 0:64], sSgrow[:, 64:128], op=ALU.add
        )
        nc.vector.tensor_tensor(sSgf, sSgf, krws[:, g, :], op=ALU.mult)
        nc.vector.tensor_tensor(sSgf, sSgf, pmr, op=ALU.add)
        x21in = sm.tile([1, 64], F32, tag="x21in")
        nc.vector.scalar_tensor_tensor(
            x21in, sSgf, 1.0 / 16384.0, b3r[:, g, :], op0=ALU.mult, op1=ALU.add
        )
        x21 = _sigmoid_softmax(nc, sm, x21in, 64)
        x21d = _dup_row(nc, sm, x21, "x21d")
        v21d = sm.tile([128, 128], BF16, tag="v21d")
        nc.vector.memset(v21d, 0.0)
        _bcast_diag_bf16(
            nc, tp, x21d, one1, v21d[0:64, 0:64], v21d[64:128, 64:128]
        )

        # ---- phase C: weights = x11@x2 + (x11*k)@Sg + x21@x1;
        #      out = feat*sig(w); then per-chunk y DMA, cascade add and
        #      bf16 shadow refresh for the next group (fully pipelined) ----
        gc0 = g * C
        for cj in range(8):
            jb = 8 * cj
            rsl = slice(jb, jb + 8)
            pwt = pw.tile([128, 1024], F32, tag="pwts")
            for h in range(2):
                osl = slice(512 * h, 512 * h + 512)
                hsl = slice(jb + 4 * h, jb + 4 * h + 4)
                nc.tensor.matmul(
                    pwt[:, osl],
                    v11d[:, g, :],
                    t_x2[:, hsl, :],
                    start=True,
                    stop=False,
                    tile_position=(0, 0),
                )
                nc.tensor.matmul(
                    pwt[:, osl],
                    x11kd[:, g, :],
                    t_sgb[:, hsl, :],
                    start=False,
                    stop=False,
                    tile_position=(0, 0),
                )
                nc.tensor.matmul(
                    pwt[:, osl],
                    v21d[:, :],
                    t_x1[:, hsl, :],
                    start=False,
                    stop=True,
                    tile_position=(0, 0),
                )
            sw_c = sgw.tile([128, 1024], BF16, tag="sw_c")
            nc.scalar.activation(
                sw_c, pwt, AF.Sigmoid, bias=cb3r[:, g : g + 1], scale=1.0
            )
            fsl = (slice(0, 128), slice(1 + jb, 9 + jb), slice(1, 129))
            eng = nc.vector
            eng.tensor_tensor(
                t_feat[fsl],
                t_feat[fsl],
                sw_c.rearrange("p (a b) -> p a b", a=8),
                op=ALU.mult,
            )
            # y chunk out (both parities)
            nc.sync.dma_start(
                out=y[gc0 : gc0 + 64, 2 * jb : 2 * jb + 16 : 2, :],
                in_=t_feat[0:64, 1 + jb : 9 + jb, 1:129],
            )
            nc.sync.dma_start(
                out=y[gc0 : gc0 + 64, 2 * jb + 1 : 2 * jb + 16 : 2, :],
                in_=t_feat[64:128, 1 + jb : 9 + jb, 1:129],
            )
            if g + 1 < G:
                # cascade add + bf16 shadow refresh for next group
                eng2 = nc.gpsimd
                eng2.tensor_tensor(
                    t_feat[fsl], t_feat[fsl], t_gx[:, rsl, :], op=ALU.add
                )
                nc.scalar.copy(t_featb[fsl], t_feat[fsl])

    return nc


_CACHE = {}


def _get_nc(split=True):
    if "nc" not in _CACHE:
        from contextlib import ExitStack

        nc = bacc.Bacc(
            "TRN2", target_bir_lowering=False, debug=False, num_devices=8
        )
        with tile.TileContext(nc) as tc:
            with ExitStack() as ctx:
                build_kernel(nc, tc, ctx)
        nc.compile()
        _CACHE["nc"] = nc
    return _CACHE["nc"]


def kernel(x, w1, b1, w3, b3, gnw, gnb):
    nc = _get_nc()
    from concourse.bass_utils import run_bass_kernel_spmd

    x = np.ascontiguousarray(np.asarray(x, dtype=np.float32))
    params = {
        "w1": np.ascontiguousarray(np.asarray(w1, np.float32)),
        "b1": np.ascontiguousarray(np.asarray(b1, np.float32)),
        "w3": np.ascontiguousarray(np.asarray(w3, np.float32)),
        "b3": np.ascontiguousarray(np.asarray(b3, np.float32)),
        "gnw": np.ascontiguousarray(np.asarray(gnw, np.float32)),
        "gnb": np.ascontiguousarray(np.asarray(gnb, np.float32)),
    }
    in_maps = [dict(params, x=np.ascontiguousarray(x[i])) for i in range(8)]
    res = run_bass_kernel_spmd(nc, in_maps, list(range(8)))
    out = np.stack([res.results[i]["y"] for i in range(8)], axis=0)
    return out
